# revision 1
# baseline (speedup 1.0000x reference)
"""ColorHistogramLoss Trainium2 kernel.

Math: reference soft-histogram weight for pixel x and bin k is
    w = exp(-(x - c_k)^2 / (2 sigma^2)),  sigma = bin_width = 1/64, c_k = (k+0.5)/64
In bin units u = 64x:  w = exp(-(u - (k+0.5))^2 / 2).
With y = x - 0.5 (exact in fp32) and e_k = (k+0.5) - 32:
    t = 64y - e_k,   t^2/2 = 2048 y^2 - 64 e_k y + e_k^2/2
So per (pixel, bin):
    w = Exp( -(2048 y^2 - 64 e_k y) - e_k^2/2 )
The quadratic form rides the TensorEngine as a K=4 constant-stationary matmul
(rows = [yA, yA^2, yB, yB^2] for two images packed on 128 PSUM partitions =
2 x 64 bins), then a single ScalarEngine Exp pass with per-partition bias
-e_k^2/2 and fused accum_out produces per-chunk bin sums.  Host folds the
per-chunk partials in fp64, cumsums, normalizes, and takes the L1 mean.

Sharding: each of the 8 cores processes a 1/8 pixel-slice of all 24 images
(12 pred + 12 target); partial histogram sums are combined on host.
"""

import os

import numpy as np

N_CORES = 8
B, C, H, W = 4, 3, 256, 256
NIMG = 2 * B * C          # 24 images (12 pred + 12 target)
NPX = H * W               # 65536 pixels / image
PXC = NPX // N_CORES      # 8192 pixels / image / core
NPAIR = NIMG // 2         # 12 image pairs packed per matmul column-block
CHUNK = 512               # pixels per matmul (f32 moving-operand max)
ACHUNK = 1024             # pixels per ACT op (2 PSUM banks)
NCH = PXC // ACHUNK       # 8 ACT chunks per pair per core
NCOL = NPAIR * NCH        # 96 accumulator columns
BINS = 64
WIDE_F = NIMG * PXC // 128  # 1536 free-dim of the wide prep layout

_CACHE = {}


def _consts():
    k = np.arange(128) % 64
    c = k + 0.5
    e = c - 32.0
    stat = np.zeros((NPAIR, 2 * NIMG, 128), np.float32)
    for j in range(NPAIR):
        stat[j, 4 * j + 0, :64] = -64.0 * e[:64]
        stat[j, 4 * j + 1, :64] = 2048.0
        stat[j, 4 * j + 2, 64:] = -64.0 * e[64:]
        stat[j, 4 * j + 3, 64:] = 2048.0
    biasd = (-(e * e) / 2.0).astype(np.float32).reshape(128, 1)
    return stat, biasd


def _build():
    import concourse.bacc as bacc
    import concourse.tile as tile
    import concourse.mybir as mybir

    f32 = mybir.dt.float32
    nc = bacc.Bacc("TRN2", target_bir_lowering=False, debug=False,
                   num_devices=N_CORES)

    xin = nc.dram_tensor("xin", [NIMG, PXC], f32, kind="ExternalInput")
    stat = nc.dram_tensor("stat", [NPAIR, 2 * NIMG, 128], f32,
                          kind="ExternalInput")
    biasd = nc.dram_tensor("biasd", [128, 1], f32, kind="ExternalInput")
    prep = nc.dram_tensor("prep", [2 * NIMG, PXC], f32)
    out = nc.dram_tensor("acc_out", [128, NCOL], f32, kind="ExternalOutput")

    with tile.TileContext(nc) as tc:
        with (
            tc.tile_pool(name="p_const", bufs=1) as cpool,
            tc.tile_pool(name="p_wide", bufs=1) as wpool,
            tc.tile_pool(name="p_pair", bufs=1) as ppool,
            tc.tile_pool(name="p_scr", bufs=2) as spool,
            tc.tile_pool(name="p_acc", bufs=1) as apool,
            tc.tile_pool(name="p_psum", bufs=3, space="PSUM") as qpool,
        ):
            stat_t = cpool.tile([2 * NIMG, NPAIR * 128], f32)
            nc.sync.dma_start(
                out=stat_t[:].rearrange("k (j m) -> k j m", m=128),
                in_=stat[:].rearrange("j k m -> k j m"),
            )
            bias_t = cpool.tile([128, 1], f32)
            nc.sync.dma_start(out=bias_t[:], in_=biasd[:])

            # wide layout: partition p, col i*64+c  <=  xin[i, p*64+c]
            xw = wpool.tile([128, WIDE_F], f32)
            nc.sync.dma_start(
                out=xw[:].rearrange("p (i c) -> p i c", c=PXC // 128),
                in_=xin[:].rearrange("i (p c) -> p i c", p=128),
            )
            yw = wpool.tile([128, WIDE_F], f32)
            nc.vector.tensor_scalar_add(out=yw[:], in0=xw[:], scalar1=-0.5)
            ysq = wpool.tile([128, WIDE_F], f32)
            nc.vector.tensor_mul(out=ysq[:], in0=yw[:], in1=yw[:])

            # prep rows 2i = y_i, 2i+1 = y_i^2
            prep_v = prep[:].rearrange("(i two) (p c) -> two p i c",
                                       two=2, p=128)
            nc.sync.dma_start(
                out=prep_v[0],
                in_=yw[:].rearrange("p (i c) -> p i c", c=PXC // 128),
            )
            nc.sync.dma_start(
                out=prep_v[1],
                in_=ysq[:].rearrange("p (i c) -> p i c", c=PXC // 128),
            )

            acc = apool.tile([128, NCOL], f32)
            # whole prep resident: [48 partitions, 8192] = 32KB/partition
            pt = ppool.tile([2 * NIMG, PXC], f32)
            nc.sync.dma_start(out=pt[:], in_=prep[:])
            for j in range(NPAIR):
                for ch in range(NCH):
                    ps = qpool.tile([128, ACHUNK], f32, tag="ps")
                    for h in range(ACHUNK // CHUNK):
                        px0 = ACHUNK * ch + CHUNK * h
                        nc.tensor.matmul(
                            out=ps[:, CHUNK * h:CHUNK * (h + 1)],
                            lhsT=stat_t[:, 128 * j:128 * (j + 1)],
                            rhs=pt[:, px0:px0 + CHUNK],
                            start=True, stop=True,
                        )
                    scr = spool.tile([128, ACHUNK], f32, tag="scr")
                    col = NCH * j + ch
                    nc.scalar.activation(
                        out=scr[:], in_=ps[:],
                        func=mybir.ActivationFunctionType.Exp,
                        bias=bias_t[:, 0:1], scale=-1.0,
                        accum_out=acc[:, col:col + 1],
                    )
            nc.sync.dma_start(out=out[:], in_=acc[:])
    if not nc.is_finalized():
        nc.finalize()
    return nc


def kernel(pred: np.ndarray, target: np.ndarray) -> np.ndarray:
    X = np.concatenate(
        [np.asarray(pred, np.float32).reshape(B * C, NPX),
         np.asarray(target, np.float32).reshape(B * C, NPX)], axis=0)

    if "nc" not in _CACHE:
        _CACHE["nc"] = _build()
    nc = _CACHE["nc"]

    statM, biasv = _consts()
    in_maps = [
        {"xin": np.ascontiguousarray(X[:, c * PXC:(c + 1) * PXC]),
         "stat": statM, "biasd": biasv}
        for c in range(N_CORES)
    ]

    from concourse.bass_utils import run_bass_kernel_spmd
    trace = bool(int(os.environ.get("KERNEL_TRACE", "0")))
    res = run_bass_kernel_spmd(nc, in_maps, core_ids=list(range(N_CORES)),
                               trace=trace)
    if res.exec_time_ns:
        _CACHE["exec_time_ns"] = res.exec_time_ns

    A = np.stack([r["acc_out"] for r in res.results]).astype(np.float64)
    # [cores, 128, NCOL] -> per (partition, pair) sums
    M = A.reshape(N_CORES, 128, NPAIR, NCH).sum(axis=(0, 3))  # [128, 12]
    Hh = np.empty((NIMG, BINS), np.float64)
    for j in range(NPAIR):
        Hh[2 * j] = M[:64, j]
        Hh[2 * j + 1] = M[64:, j]
    cum = np.cumsum(Hh, axis=1)
    den = cum[:, -1:] + 1e-8
    cdf = cum / den
    loss = np.mean(np.abs(cdf[:B * C] - cdf[B * C:]))
    return np.array(loss, dtype=np.float32)



# revision 3
# speedup vs baseline: 1.7117x; 1.7117x over previous
"""ColorHistogramLoss Trainium2 kernel (v2: bf16 single-stream matmul).

Math: reference soft-histogram weight for pixel x and bin k is
    w = exp(-(x - c_k)^2 / (2 sigma^2)),  sigma = bin_width = 1/64, c_k = (k+0.5)/64
In bin units u = 64x, with y = x - 0.5 (exact in fp32) and e_k = (k+0.5) - 32:
    t = 64y - e_k,   t^2/2 = 2048 y^2 - 64 e_k y + e_k^2/2
Each image's per-pixel features y and s = y^2 are split exactly into three
bf16 terms (3 x 8 mantissa bits >= fp32's 24), so the quadratic form rides
the TensorEngine as a K=12 bf16 matmul (rows = [yh,ym,yl,sh,sm,sl] for two
images packed on 128 PSUM partitions = 2 x 64 bins) at 1 cycle/column with
ONE constant stationary. A ScalarEngine Exp pass per 2048-column PSUM chunk
(bias -e_k^2/2, fused accum_out) produces per-chunk bin sums. Host folds the
partials in fp64, cumsums, normalizes, takes the L1 mean.

Sharding: each of the 8 cores processes a 1/8 pixel-slice of all 24 images
(12 pred + 12 target); partial histogram sums are combined on host.
"""

import os

import numpy as np

N_CORES = 8
B, C, H, W = 4, 3, 256, 256
NIMG = 2 * B * C          # 24 images (12 pred + 12 target)
NPX = H * W               # 65536 pixels / image
PXC = NPX // N_CORES      # 8192 pixels / image / core
NPAIR = NIMG // 2         # 12 image pairs packed per matmul column-block
CHUNK = 512               # pixels per matmul (PSUM bank limit in f32 out)
ACH = 2048                # pixels per ACT op (4 PSUM banks)
NCH = PXC // ACH          # 4 ACT chunks per pair per core
NCOL = NPAIR * NCH        # 48 accumulator columns
BINS = 64
CPL = PXC // 128          # 64 columns per lane in the wide layout
WF = NIMG * CPL           # 1536 wide free-dim
NGRP = 4                  # image groups for pipelining
GIMG = NIMG // NGRP       # 6 images per group
GPAIR = GIMG // 2         # 3 pairs per group
GW = GIMG * CPL           # 384 wide cols per group

_CACHE = {}


def _consts():
    import ml_dtypes
    e = (np.arange(64) + 0.5 - 32.0).astype(np.float32)
    stat = np.zeros((12, 128), np.float32)
    stat[0:3, :64] = -64.0 * e
    stat[3:6, :64] = 2048.0
    stat[6:9, 64:] = -64.0 * e
    stat[9:12, 64:] = 2048.0
    statw = stat.astype(ml_dtypes.bfloat16)
    ee = np.concatenate([e, e])
    biasd = (-(ee * ee) / 2.0).astype(np.float32).reshape(128, 1)
    return statw, biasd


def _build():
    import concourse.bacc as bacc
    import concourse.tile as tile
    import concourse.mybir as mybir

    f32 = mybir.dt.float32
    bf16 = mybir.dt.bfloat16
    nc = bacc.Bacc("TRN2", target_bir_lowering=False, debug=False,
                   num_devices=N_CORES)

    xin = nc.dram_tensor("xin", [NIMG, PXC], f32, kind="ExternalInput")
    statw = nc.dram_tensor("statw", [12, 128], bf16, kind="ExternalInput")
    biasd = nc.dram_tensor("biasd", [128, 1], f32, kind="ExternalInput")
    # one scratch DRAM tensor per image group so pair-reads only depend on
    # their own group's writes
    preps = [nc.dram_tensor(f"prep{g}", [12, GPAIR * PXC], bf16)
             for g in range(NGRP)]
    out = nc.dram_tensor("acc_out", [128, NCOL], f32, kind="ExternalOutput")

    with tile.TileContext(nc) as tc:
        with (
            tc.tile_pool(name="p_const", bufs=1) as cpool,
            tc.tile_pool(name="p_wide", bufs=1) as wpool,
            tc.tile_pool(name="p_pair", bufs=3) as ppool,
            tc.tile_pool(name="p_scr", bufs=2) as spool,
            tc.tile_pool(name="p_acc", bufs=1) as apool,
            tc.tile_pool(name="p_psum", bufs=2, space="PSUM") as qpool,
        ):
            stat_t = cpool.tile([12, 128], bf16)
            nc.sync.dma_start(out=stat_t[:], in_=statw[:])
            bias_t = cpool.tile([128, 1], f32)
            nc.sync.dma_start(out=bias_t[:], in_=biasd[:])

            # wide layout: partition p, col i*64+c  <=  xin[i, p*64+c]
            xw = wpool.tile([128, WF], f32)
            y32 = wpool.tile([128, WF], f32)
            s32 = wpool.tile([128, WF], f32)
            t1 = wpool.tile([128, WF], f32)
            t2 = wpool.tile([128, WF], f32)
            # six bf16 feature tiles: yh ym yl sh sm sl
            feats = [wpool.tile([128, WF], bf16, name=f"feat{i}")
                     for i in range(6)]

            acc = apool.tile([128, NCOL], f32)

            for g in range(NGRP):
                lo, hi = g * GW, (g + 1) * GW
                nc.sync.dma_start(
                    out=xw[:, lo:hi].rearrange("p (i c) -> p i c", c=CPL),
                    in_=xin[g * GIMG:(g + 1) * GIMG].rearrange(
                        "i (p c) -> p i c", p=128),
                )
                sl_ = (slice(None), slice(lo, hi))
                yv, sv, t1v, t2v = y32[sl_], s32[sl_], t1[sl_], t2[sl_]
                nc.vector.tensor_scalar_add(out=yv, in0=xw[sl_], scalar1=-0.5)
                nc.vector.tensor_mul(out=sv, in0=yv, in1=yv)
                # exact 3-way bf16 splits of y and s
                nc.vector.tensor_copy(out=feats[0][sl_], in_=yv)
                nc.vector.tensor_sub(out=t1v, in0=yv, in1=feats[0][sl_])
                nc.vector.tensor_copy(out=feats[1][sl_], in_=t1v)
                nc.vector.tensor_sub(out=t2v, in0=t1v, in1=feats[1][sl_])
                nc.vector.tensor_copy(out=feats[2][sl_], in_=t2v)
                nc.vector.tensor_copy(out=feats[3][sl_], in_=sv)
                nc.vector.tensor_sub(out=t1v, in0=sv, in1=feats[3][sl_])
                nc.vector.tensor_copy(out=feats[4][sl_], in_=t1v)
                nc.vector.tensor_sub(out=t2v, in0=t1v, in1=feats[4][sl_])
                nc.vector.tensor_copy(out=feats[5][sl_], in_=t2v)

                # write transposed bf16 features to DRAM: row f + 6*b holds
                # feature f of image 2j+b, pixels ordered (lane, col)
                for il in range(GIMG):
                    i = g * GIMG + il
                    jj, bb = il // 2, il % 2
                    for f in range(6):
                        nc.sync.dma_start(
                            out=preps[g][f + 6 * bb,
                                         jj * PXC:(jj + 1) * PXC].rearrange(
                                "(p c) -> p c", p=128),
                            in_=feats[f][:, i * CPL:(i + 1) * CPL],
                        )

            for j in range(NPAIR):
                g, jj = j // GPAIR, j % GPAIR
                pt = ppool.tile([12, PXC], bf16, tag="pt")
                nc.sync.dma_start(
                    out=pt[:], in_=preps[g][:, jj * PXC:(jj + 1) * PXC])
                for ch in range(NCH):
                    ps = qpool.tile([128, ACH], f32, tag="ps")
                    for h in range(ACH // CHUNK):
                        px0 = ACH * ch + CHUNK * h
                        nc.tensor.matmul(
                            out=ps[:, CHUNK * h:CHUNK * (h + 1)],
                            lhsT=stat_t[:],
                            rhs=pt[:, px0:px0 + CHUNK],
                            start=True, stop=True,
                        )
                    scr = spool.tile([128, ACH], bf16, tag="scr")
                    col = NCH * j + ch
                    nc.scalar.activation(
                        out=scr[:], in_=ps[:],
                        func=mybir.ActivationFunctionType.Exp,
                        bias=bias_t[:, 0:1], scale=-1.0,
                        accum_out=acc[:, col:col + 1],
                    )
            nc.sync.dma_start(out=out[:], in_=acc[:])
    if not nc.is_finalized():
        nc.finalize()
    return nc


def kernel(pred: np.ndarray, target: np.ndarray) -> np.ndarray:
    X = np.concatenate(
        [np.asarray(pred, np.float32).reshape(B * C, NPX),
         np.asarray(target, np.float32).reshape(B * C, NPX)], axis=0)

    if "nc" not in _CACHE:
        _CACHE["nc"] = _build()
    nc = _CACHE["nc"]

    statw, biasv = _consts()
    in_maps = [
        {"xin": np.ascontiguousarray(X[:, c * PXC:(c + 1) * PXC]),
         "statw": statw, "biasd": biasv}
        for c in range(N_CORES)
    ]

    from concourse.bass_utils import run_bass_kernel_spmd
    trace = bool(int(os.environ.get("KERNEL_TRACE", "0")))
    res = run_bass_kernel_spmd(nc, in_maps, core_ids=list(range(N_CORES)),
                               trace=trace)
    if res.exec_time_ns:
        _CACHE["exec_time_ns"] = res.exec_time_ns

    A = np.stack([r["acc_out"] for r in res.results]).astype(np.float64)
    # [cores, 128, NCOL] -> per (partition, pair) sums
    M = A.reshape(N_CORES, 128, NPAIR, NCH).sum(axis=(0, 3))  # [128, 12]
    Hh = np.empty((NIMG, BINS), np.float64)
    for j in range(NPAIR):
        Hh[2 * j] = M[:64, j]
        Hh[2 * j + 1] = M[64:, j]
    cum = np.cumsum(Hh, axis=1)
    den = cum[:, -1:] + 1e-8
    cdf = cum / den
    loss = np.mean(np.abs(cdf[:B * C] - cdf[B * C:]))
    return np.array(loss, dtype=np.float32)


# revision 10
# speedup vs baseline: 2.3175x; 1.3540x over previous
"""ColorHistogramLoss Trainium2 kernel (v2: bf16 single-stream matmul).

Math: reference soft-histogram weight for pixel x and bin k is
    w = exp(-(x - c_k)^2 / (2 sigma^2)),  sigma = bin_width = 1/64, c_k = (k+0.5)/64
In bin units u = 64x, with y = x - 0.5 (exact in fp32) and e_k = (k+0.5) - 32:
    t = 64y - e_k,   t^2/2 = 2048 y^2 - 64 e_k y + e_k^2/2
Each image's per-pixel features y and s = y^2 are split exactly into three
bf16 terms (3 x 8 mantissa bits >= fp32's 24), so the quadratic form rides
the TensorEngine as a K=12 bf16 matmul (rows = [yh,ym,yl,sh,sm,sl] for two
images packed on 128 PSUM partitions = 2 x 64 bins) at 1 cycle/column with
ONE constant stationary. A ScalarEngine Exp pass per 2048-column PSUM chunk
(bias -e_k^2/2, fused accum_out) produces per-chunk bin sums. Host folds the
partials in fp64, cumsums, normalizes, takes the L1 mean.

Sharding: each of the 8 cores processes a 1/8 pixel-slice of all 24 images
(12 pred + 12 target); partial histogram sums are combined on host.
"""

import os

import numpy as np

N_CORES = 8
B, C, H, W = 4, 3, 256, 256
NIMG = 2 * B * C          # 24 images (12 pred + 12 target)
NPX = H * W               # 65536 pixels / image
PXC = NPX // N_CORES      # 8192 pixels / image / core
NPAIR = NIMG // 2         # 12 image pairs packed per matmul column-block
CHUNK = 512               # pixels per matmul (PSUM bank limit in f32 out)
ACH = 2048                # pixels per ACT op (4 PSUM banks)
NCH = PXC // ACH          # 4 ACT chunks per pair per core
NCOL = NPAIR * NCH        # 48 accumulator columns
BINS = 64
CPL = PXC // 128          # 64 columns per lane in the wide layout
WF = NIMG * CPL           # 1536 wide free-dim
NGRP = 4                  # image groups for pipelining
GIMG = NIMG // NGRP       # 6 images per group
GPAIR = GIMG // 2         # 3 pairs per group
GW = GIMG * CPL           # 384 wide cols per group

_CACHE = {}


def _consts():
    import ml_dtypes
    e = (np.arange(64) + 0.5 - 32.0).astype(np.float32)
    stat = np.zeros((12, 128), np.float32)
    stat[0:3, :64] = -64.0 * e
    stat[3:6, :64] = 2048.0
    stat[6:9, 64:] = -64.0 * e
    stat[9:12, 64:] = 2048.0
    statw = stat.astype(ml_dtypes.bfloat16)
    ee = np.concatenate([e, e])
    biasd = (-(ee * ee) / 2.0).astype(np.float32).reshape(128, 1)
    return statw, biasd


def _build():
    import concourse.bacc as bacc
    import concourse.tile as tile
    import concourse.mybir as mybir

    f32 = mybir.dt.float32
    bf16 = mybir.dt.bfloat16
    nc = bacc.Bacc("TRN2", target_bir_lowering=False, debug=False,
                   num_devices=N_CORES)

    xin = nc.dram_tensor("xin", [NIMG, PXC], f32, kind="ExternalInput")
    statw = nc.dram_tensor("statw", [12, 128], bf16, kind="ExternalInput")
    biasd = nc.dram_tensor("biasd", [128, 1], f32, kind="ExternalInput")
    # one scratch DRAM tensor per image group so pair-reads only depend on
    # their own group's writes; layout mirrors the SBUF feature tile:
    # prep[p, il*384 + f*64 + c] (image-major, 6 features of 64 cols each)
    preps = [nc.dram_tensor(f"prep{g}", [128, GIMG * 6 * CPL], bf16)
             for g in range(NGRP)]
    out = nc.dram_tensor("acc_out", [128, NCOL], f32, kind="ExternalOutput")

    with tile.TileContext(nc) as tc:
        with (
            tc.tile_pool(name="p_const", bufs=1) as cpool,
            tc.tile_pool(name="p_wide", bufs=1) as wpool,
            tc.tile_pool(name="p_pair", bufs=3) as ppool,
            tc.tile_pool(name="p_scr", bufs=2) as spool,
            tc.tile_pool(name="p_acc", bufs=1) as apool,
            tc.tile_pool(name="p_psum", bufs=2, space="PSUM") as qpool,
        ):
            stat_t = cpool.tile([12, 128], bf16)
            nc.sync.dma_start(out=stat_t[:], in_=statw[:])
            bias_t = cpool.tile([128, 1], f32)
            nc.sync.dma_start(out=bias_t[:], in_=biasd[:])

            # wide layout: partition p, col i*64+c  <=  xin[i, p*64+c]
            xw = wpool.tile([128, WF], f32)
            y32 = wpool.tile([128, WF], f32)
            s32 = wpool.tile([128, WF], f32)
            t1 = wpool.tile([128, WF], f32)
            t2 = wpool.tile([128, WF], f32)
            # one combined bf16 feature tile, image-major:
            # featall[p, i*384 + f*64 + c] = feature f of image i, col c.
            # With i = 2*jj + b the byte offset of (jj, b, f) is
            # jj*768 + (6*b + f)*64 — rows r = 6b+f of the matmul moving
            # operand read out linearly at stride 64.
            featall = wpool.tile([128, 6 * WF], bf16)
            fview = featall[:].rearrange("p (i f c) -> p f i c", f=6, c=CPL)

            acc = apool.tile([128, NCOL], f32)

            for g in range(NGRP):
                lo, hi = g * GW, (g + 1) * GW
                nc.sync.dma_start(
                    out=xw[:, lo:hi].rearrange("p (i c) -> p i c", c=CPL),
                    in_=xin[g * GIMG:(g + 1) * GIMG].rearrange(
                        "i (p c) -> p i c", p=128),
                )
                def v3(t):
                    return t[:, lo:hi].rearrange("p (i c) -> p i c", c=CPL)
                yv, sv, t1v, t2v = v3(y32), v3(s32), v3(t1), v3(t2)
                nc.vector.tensor_scalar_add(out=yv, in0=v3(xw), scalar1=-0.5)
                nc.vector.tensor_mul(out=sv, in0=yv, in1=yv)
                # exact 3-way bf16 splits of y and s
                fs = [fview[:, f, g * GIMG:(g + 1) * GIMG, :]
                      for f in range(6)]
                nc.vector.tensor_copy(out=fs[0], in_=yv)
                nc.vector.tensor_sub(out=t1v, in0=yv, in1=fs[0])
                nc.vector.tensor_copy(out=fs[1], in_=t1v)
                nc.vector.tensor_sub(out=t2v, in0=t1v, in1=fs[1])
                nc.vector.tensor_copy(out=fs[2], in_=t2v)
                nc.vector.tensor_copy(out=fs[3], in_=sv)
                nc.vector.tensor_sub(out=t1v, in0=sv, in1=fs[3])
                nc.vector.tensor_copy(out=fs[4], in_=t1v)
                nc.vector.tensor_sub(out=t2v, in0=t1v, in1=fs[4])
                nc.vector.tensor_copy(out=fs[5], in_=t2v)

                # plain 2-dim copy to DRAM; the transpose happens on the
                # per-pair read side (3-dim AP)
                gw6 = GIMG * 6 * CPL
                nc.sync.dma_start(
                    out=preps[g][:],
                    in_=featall[:, g * gw6:(g + 1) * gw6])

            for j in range(NPAIR):
                g, jj = j // GPAIR, j % GPAIR
                pt = ppool.tile([12, PXC], bf16, tag="pt")
                # transposing read: row r=6b+f <- prep[p, jj*768 + r*64 + c]
                nc.sync.dma_start(
                    out=pt[:].rearrange("r (p c) -> r p c", p=128),
                    in_=preps[g][:].rearrange(
                        "p (jj r c) -> jj r p c", jj=GPAIR, r=12)[jj],
                )
                for ch in range(NCH):
                    ps = qpool.tile([128, ACH], f32, tag="ps")
                    for h in range(ACH // CHUNK):
                        px0 = ACH * ch + CHUNK * h
                        nc.tensor.matmul(
                            out=ps[:, CHUNK * h:CHUNK * (h + 1)],
                            lhsT=stat_t[:],
                            rhs=pt[:, px0:px0 + CHUNK],
                            start=True, stop=True,
                        )
                    scr = spool.tile([128, ACH], bf16, tag="scr")
                    col = NCH * j + ch
                    nc.scalar.activation(
                        out=scr[:], in_=ps[:],
                        func=mybir.ActivationFunctionType.Exp,
                        bias=bias_t[:, 0:1], scale=-1.0,
                        accum_out=acc[:, col:col + 1],
                    )
            nc.sync.dma_start(out=out[:], in_=acc[:])
    if not nc.is_finalized():
        nc.finalize()
    return nc


def kernel(pred: np.ndarray, target: np.ndarray) -> np.ndarray:
    X = np.concatenate(
        [np.asarray(pred, np.float32).reshape(B * C, NPX),
         np.asarray(target, np.float32).reshape(B * C, NPX)], axis=0)

    if "nc" not in _CACHE:
        _CACHE["nc"] = _build()
    nc = _CACHE["nc"]

    statw, biasv = _consts()
    in_maps = [
        {"xin": np.ascontiguousarray(X[:, c * PXC:(c + 1) * PXC]),
         "statw": statw, "biasd": biasv}
        for c in range(N_CORES)
    ]

    from concourse.bass_utils import run_bass_kernel_spmd
    trace = bool(int(os.environ.get("KERNEL_TRACE", "0")))
    res = run_bass_kernel_spmd(nc, in_maps, core_ids=list(range(N_CORES)),
                               trace=trace)
    if res.exec_time_ns:
        _CACHE["exec_time_ns"] = res.exec_time_ns

    A = np.stack([r["acc_out"] for r in res.results]).astype(np.float64)
    # [cores, 128, NCOL] -> per (partition, pair) sums
    M = A.reshape(N_CORES, 128, NPAIR, NCH).sum(axis=(0, 3))  # [128, 12]
    Hh = np.empty((NIMG, BINS), np.float64)
    for j in range(NPAIR):
        Hh[2 * j] = M[:64, j]
        Hh[2 * j + 1] = M[64:, j]
    cum = np.cumsum(Hh, axis=1)
    den = cum[:, -1:] + 1e-8
    cdf = cum / den
    loss = np.mean(np.abs(cdf[:B * C] - cdf[B * C:]))
    return np.array(loss, dtype=np.float32)
